# revision 1
# baseline (speedup 1.0000x reference)
"""MHA kernel for Trainium2, 8 NeuronCores.

Sharding: (batch, query-half) -> 8 shards. Core c handles batch c//2,
query rows (c%2)*1024:(c%2+1)*1024. Each core computes all 16 heads for its
1024 query rows; K/V projections for its batch are computed locally
(duplicated across the 2 cores sharing a batch). Output rows are disjoint
across cores -> no collectives.

Host-side marshaling (part of sharding): X slices are transposed and cast to
bf16 ([D, S] layout) so the device does zero transposes of X; weights are
pre-cast/pre-packed into the SBUF layouts the matmuls want. All device DMAs
are large row-contiguous loads.

Per-core compute (bf16 matmuls, f32 PSUM):
  per head-pair hp: qT/kT/vT [128 = pair-stacked 64+64, S] via 8-chunk
  accumulated matmuls over D; biases folded into the PSUM->SBUF copy as
  per-partition scalars (DVE tensor_scalar_add). v natural [s, 130] via PE
  transpose of vT blocks (ones cols interleaved for softmax rowsums).
  scores^T per head via K=64 matmuls (head pair concurrent in PE row groups)
  E = exp(scores^T) on ScalarE; z'^T accumulated with the ones-column trick;
  z^T = z'^T[:64] * recip(z'^T[64]) (GpSimd partition-broadcast from base 0).
  out = z^T.T @ Wo + bo (bo added via DVE from a pre-broadcast tile).
"""

import os

import numpy as np
import ml_dtypes



import concourse.bass as bass
import concourse.tile as tile
from concourse import bacc, mybir
from concourse.bass import ds, ts
from concourse.bass_utils import run_bass_kernel_spmd

B, S, D = 4, 2048, 1024
H, DK, DV = 16, 64, 64
N_CORES = 8
SQ = S // 2  # query rows per core
P = 128
NHP = H // 2  # head pairs
F32 = mybir.dt.float32
BF = mybir.dt.bfloat16
EXP = mybir.ActivationFunctionType.Exp


def build_kernel(nc, tc, VARIANT=""):
    # pre-transposed bf16 inputs [D, S*] (host-marshaled)
    xqT_d = nc.declare_dram_parameter("xqT", [D, SQ], BF, isOutput=False).ap()
    xkT_d = nc.declare_dram_parameter("xkT", [D, S], BF, isOutput=False).ap()
    xvT_d = nc.declare_dram_parameter("xvT", [D, S], BF, isOutput=False).ap()
    # weights pre-packed [128, 8 dchunk, 1024 (h k)] bf16
    wq_d = nc.declare_dram_parameter("wq", [P, 8, H * DK], BF, isOutput=False).ap()
    wk_d = nc.declare_dram_parameter("wk", [P, 8, H * DK], BF, isOutput=False).ap()
    wv_d = nc.declare_dram_parameter("wv", [P, 8, H * DV], BF, isOutput=False).ap()
    # biases pre-packed [128 (pair-stacked), 8 hp] f32
    bq_d = nc.declare_dram_parameter("bq", [P, NHP], F32, isOutput=False).ap()
    bk_d = nc.declare_dram_parameter("bk", [P, NHP], F32, isOutput=False).ap()
    bvr_d = nc.declare_dram_parameter("bvr", [1, H * DV], BF, isOutput=False).ap()
    # Wo pre-packed [128, 8 fchunk, 1024 dout] bf16; bo [1, D] f32
    wo_d = nc.declare_dram_parameter("wo", [P, 8, D], BF, isOutput=False).ap()
    bo_d = nc.declare_dram_parameter("bo", [1, D], F32, isOutput=False).ap()
    out = nc.declare_dram_parameter("out", [SQ, D], F32, isOutput=True).ap()

    import contextlib

    ctx = contextlib.ExitStack()
    with ctx:
        consts = ctx.enter_context(tc.tile_pool(name="consts", bufs=1))
        wpool = ctx.enter_context(tc.tile_pool(name="wpool", bufs=1))
        xtp = ctx.enter_context(tc.tile_pool(name="xtp", bufs=1))
        ztp = ctx.enter_context(tc.tile_pool(name="ztp", bufs=1))
        qkv = ctx.enter_context(tc.tile_pool(name="qkv", bufs=2))
        epool = ctx.enter_context(tc.tile_pool(name="epool", bufs=3))
        rpool = ctx.enter_context(tc.tile_pool(name="rpool", bufs=2))
        opool = ctx.enter_context(tc.tile_pool(name="opool", bufs=2))
        wopool = ctx.enter_context(tc.tile_pool(name="wopool", bufs=1))
        pp = ctx.enter_context(tc.tile_pool(name="pp", bufs=2, space=bass.MemorySpace.PSUM))
        sp = ctx.enter_context(tc.tile_pool(name="sp", bufs=2, space=bass.MemorySpace.PSUM))
        zp = ctx.enter_context(tc.tile_pool(name="zp", bufs=2, space=bass.MemorySpace.PSUM))

        # ---- constants / weights ----
        bqc = consts.tile([P, NHP], F32, tag="bqc")
        bkc = consts.tile([P, NHP], F32, tag="bkc")
        nc.gpsimd.dma_start(out=bqc[:, :], in_=bq_d[:, :])
        nc.gpsimd.dma_start(out=bkc[:, :], in_=bk_d[:, :])
        bvr = consts.tile([1, H * DV], BF, tag="bvr")
        nc.gpsimd.dma_start(out=bvr[0:1, :], in_=bvr_d[0:1, :])
        bvb = consts.tile([P, H * DV], BF, tag="bvb")
        nc.gpsimd.partition_broadcast(bvb[:, :], bvr[0:1, :])
        bo_row16 = consts.tile([1, D], BF, tag="bo_row16")
        nc.gpsimd.dma_start(out=bo_row16[0:1, :], in_=bo_d[0:1, :])
        bo_bc = consts.tile([P, D], BF, tag="bo_bc")
        nc.gpsimd.partition_broadcast(bo_bc[:, :], bo_row16[0:1, :])

        wq_sb = wpool.tile([P, 8, H * DK], BF, tag="wq_sb")
        wk_sb = wpool.tile([P, 8, H * DK], BF, tag="wk_sb")
        wv_sb = wpool.tile([P, 8, H * DV], BF, tag="wv_sb")

        # ---- xT loads: [128, 8 dchunk, S] bf16 ----
        xqT = xtp.tile([P, 8, SQ], BF, tag="xqT")
        xkT = xtp.tile([P, 8, S], BF, tag="xkT")
        xvT = xtp.tile([P, 8, S], BF, tag="xvT")
        # whole-tensor weight loads (contiguous; 1 DMA each, few descriptors)
        nc.gpsimd.dma_start(out=wq_sb[:, :, :], in_=wq_d[:, :, :])
        # x loads: full [128, S] chunks (1 descriptor per partition-row; a
        # half-width slice would cost the SAME descriptor count), spread
        # across all three queues in first-use order (xqT gates the ramp)
        for dc in range(6):
            eng = nc.sync if dc % 2 == 0 else nc.scalar
            eng.dma_start(out=xqT[:, dc, :], in_=xqT_d[ds(dc * P, P), :])
        nc.gpsimd.dma_start(out=xqT[:, 6, :], in_=xqT_d[ds(6 * P, P), :])
        nc.gpsimd.dma_start(out=xqT[:, 7, :], in_=xqT_d[ds(7 * P, P), :])
        nc.gpsimd.dma_start(out=wk_sb[:, :, :], in_=wk_d[:, :, :])
        for dc in range(6):
            eng = nc.sync if dc % 2 == 0 else nc.scalar
            eng.dma_start(out=xkT[:, dc, :], in_=xkT_d[ds(dc * P, P), :])
        nc.gpsimd.dma_start(out=xkT[:, 6, :], in_=xkT_d[ds(6 * P, P), :])
        nc.gpsimd.dma_start(out=xkT[:, 7, :], in_=xkT_d[ds(7 * P, P), :])
        nc.gpsimd.dma_start(out=wv_sb[:, :, :], in_=wv_d[:, :, :])
        for dc in range(8):
            eng = (nc.gpsimd, nc.sync, nc.scalar)[dc % 3]
            eng.dma_start(out=xvT[:, dc, :], in_=xvT_d[ds(dc * P, P), :])

        wo_sb0 = wopool.tile([P, 8, 512], BF, tag="wo_sb")
        for fc in range(8):
            nc.gpsimd.dma_start(out=wo_sb0[:, fc, :], in_=wo_d[:, fc, 0:512])

        # z^T accumulator: [128 = dv(h0)|dv(h1), 8 head-pairs, 1024 q]
        zT = ztp.tile([P, NHP, SQ], BF, tag="zT")

        def outproj_step(qc, dt, wo_sb):
            def emit():
                ps = pp.tile([P, 512], F32, tag="proj")
                for fc in range(8):
                    nc.tensor.matmul(ps[:, :], zT[:, fc, ts(qc, P)],
                                     wo_sb[:, fc, :],
                                     start=(fc == 0), stop=(fc == 7))
                o_t = opool.tile([P, 512], F32, tag="o")
                nc.vector.tensor_add(o_t[:, :], ps[:, :], bo_bc[:, ts(dt, 512)])
                nc.sync.dma_start(out=out[ts(qc, P), ts(dt, 512)], in_=o_t[:, :])
            return emit

        # qc 0-3 / dt 0 depend only on qt0 rows of zT -> weavable into the
        # last head-pair's qt1 attention (which has no next-proj to weave)
        op_early = [outproj_step(qc, 0, wo_sb0) for qc in range(4)]

        # ---- main loop: quads of head-pairs (v projected per quad) ----
        vqp = ctx.enter_context(tc.tile_pool(name="vqp", bufs=1))
        NJ = S // P  # 16 k-chunks
        if VARIANT == "half_att":
            NJ = 8
        def emit_vq(q4):
            # v natural for 4 head-pairs: [128 s, 16 sc, 4 hp, 130]
            v_q = vqp.tile([P, S // P, 4, 130], BF, tag="v_q")
            vv = v_q.rearrange("p s h (a c) -> p s h a c", a=2)
            nc.vector.memset(vv[:, :, :, :, 64:65], 1.0)
            q4sl = ds(q4 * 512, 512)  # 4 hp = 512 cols in (h k) layout
            for sc in range(S // P):
                ps = pp.tile([P, 512], F32, tag="proj")
                for dc in range(8):
                    nc.tensor.matmul(ps[:, :], xvT[:, dc, ts(sc, P)],
                                     wv_sb[:, dc, q4sl],
                                     start=(dc == 0), stop=(dc == 7))
                nc.vector.tensor_add(
                    vv[:, sc, :, :, 0:64],
                    ps[:, :].rearrange("p (h a c) -> p h a c", h=4, a=2),
                    bvb[:, q4sl].rearrange("p (h a c) -> p h a c", h=4, a=2),
                )
            return v_q

        for q4 in range(NHP // 4):
            for hp_i in range(4):
                hp = q4 * 4 + hp_i
                hsl = ts(hp, P)

                def make_proj_steps(hp, hsl):
                    """List of closures emitting HALF psum-groups (4 mms) of
                    the q^T/k^T projection for head-pair hp — fine-grained so
                    woven steps fit inside ACT-exp gaps without delaying zv."""
                    q_t = qkv.tile([P, SQ], BF, tag="q_t")
                    k_t = qkv.tile([P, S], BF, tag="k_t")
                    steps = []
                    state = {}

                    def half(w_sb, x_T, dst, bias, col, lo):
                        def emit():
                            if lo:
                                ps = pp.tile([P, 512], F32, tag="proj")
                                state[(dst.name, col)] = ps
                            else:
                                ps = state.pop((dst.name, col))
                            for dc in range(4):
                                d = dc if lo else dc + 4
                                nc.tensor.matmul(ps[:, :], w_sb[:, d, hsl],
                                                 x_T[:, d, ds(col, 512)],
                                                 start=(d == 0), stop=(d == 7))
                            if not lo:
                                nc.vector.tensor_scalar_add(
                                    dst[:, ds(col, 512)], ps[:, :], bias)
                        return emit

                    for qt in range(SQ // 512):
                        for lo in (True, False):
                            steps.append(half(wq_sb, xqT, q_t,
                                              bqc[:, hp:hp + 1], qt * 512, lo))
                    for st in range(S // 512):
                        for lo in (True, False):
                            steps.append(half(wk_sb, xkT, k_t,
                                              bkc[:, hp:hp + 1], st * 512, lo))
                    return q_t, k_t, steps

                if hp_i == 0 and q4 == 0:
                    q_t, k_t, steps = make_proj_steps(hp, hsl)
                    for s_ in steps:
                        s_()
                else:
                    q_t, k_t = next_qt, next_kt  # noqa: F821
                if hp_i == 0:
                    v_q = emit_vq(q4)

                # prepare next head-pair's projection steps to interleave
                next_steps = []
                if hp + 1 < NHP:
                    nhp = hp + 1
                    next_qt, next_kt, next_steps = make_proj_steps(nhp, ts(nhp, P))
                pending = list(next_steps)

                # ---- attention for the two heads of this pair ----
                for qt in range(SQ // 512):
                    qsl = ts(qt, 512)
                    ps_z0 = zp.tile([P, 512], F32, tag="z")
                    ps_z1 = zp.tile([P, 512], F32, tag="z")
                    for j2 in range(NJ // 2):
                        ps_s0 = sp.tile([P, 1024], F32, tag="s")
                        ps_s1 = sp.tile([P, 1024], F32, tag="s")
                        for i in range(2):
                            j = 2 * j2 + i
                            nc.tensor.matmul(ps_s0[:, ts(i, 512)],
                                             k_t[0:64, ts(j, P)], q_t[0:64, qsl],
                                             start=True, stop=True)
                            nc.tensor.matmul(ps_s1[:, ts(i, 512)],
                                             k_t[64:P, ts(j, P)], q_t[64:P, qsl],
                                             start=True, stop=True)
                        e0 = epool.tile([P, 1024], BF, tag="e")
                        e1 = epool.tile([P, 1024], BF, tag="e")
                        if VARIANT == "noexp":
                            nc.vector.tensor_copy(e0[:, :], ps_s0[:, :])
                            nc.vector.tensor_copy(e1[:, :], ps_s1[:, :])
                        elif VARIANT == "expcopy":
                            nc.scalar.copy(e0[:, :], ps_s0[:, :])
                            nc.scalar.copy(e1[:, :], ps_s1[:, :])
                        else:
                            nc.scalar.activation(e0[:, :], ps_s0[:, :], EXP)
                            nc.scalar.activation(e1[:, :], ps_s1[:, :], EXP)
                        # weave one projection psum-group of hp+1 in every
                        # few groups so PE has fill work while ACT exps
                        if "noweave" not in VARIANT and j2 % 2 == 1:
                            if pending:
                                pending.pop(0)()
                            elif hp == NHP - 1 and qt == 1 and op_early:
                                op_early.pop(0)()
                        for i in range(2):
                            j = 2 * j2 + i
                            nc.tensor.matmul(ps_z0[0:65, :],
                                             v_q[:, j, hp_i, 0:65],
                                             e0[:, ts(i, 512)],
                                             start=(j == 0), stop=(j == NJ - 1))
                            nc.tensor.matmul(ps_z1[0:65, :],
                                             v_q[:, j, hp_i, 65:130],
                                             e1[:, ts(i, 512)],
                                             start=(j == 0), stop=(j == NJ - 1))
                    # normalize: z = z' * recip(rowsum)
                    r0 = rpool.tile([1, 512], F32, tag="r")
                    nc.vector.reciprocal(r0[0:1, :], ps_z0[64:65, :])
                    rb0 = rpool.tile([64, 512], F32, tag="rb")
                    nc.gpsimd.partition_broadcast(rb0[:, :], r0[0:1, :])
                    nc.vector.tensor_mul(zT[0:64, hp, qsl], ps_z0[0:64, :], rb0[:, :])
                    r1 = rpool.tile([1, 512], F32, tag="r")
                    nc.vector.reciprocal(r1[0:1, :], ps_z1[64:65, :])
                    rb1 = rpool.tile([64, 512], F32, tag="rb")
                    nc.gpsimd.partition_broadcast(rb1[:, :], r1[0:1, :])
                    nc.vector.tensor_mul(zT[64:P, hp, qsl], ps_z1[0:64, :], rb1[:, :])
                for s_ in pending:
                    s_()

        # ---- output projection (remainder) ----
        for s_ in op_early:   # any not woven (e.g. half_att variants)
            s_()
        n_early = 4
        for dt in range(2):
            if dt == 0:
                wo_sb = wo_sb0
            else:
                wo_sb = wopool.tile([P, 8, 512], BF, tag="wo_sb")
                for fc in range(8):
                    nc.scalar.dma_start(out=wo_sb[:, fc, :],
                                        in_=wo_d[:, fc, ts(dt, 512)])
            for qc in range(n_early if dt == 0 else 0, SQ // P):
                ps = pp.tile([P, 512], F32, tag="proj")
                for fc in range(8):
                    nc.tensor.matmul(ps[:, :], zT[:, fc, ts(qc, P)], wo_sb[:, fc, :],
                                     start=(fc == 0), stop=(fc == 7))
                o_t = opool.tile([P, 512], F32, tag="o")
                nc.vector.tensor_add(o_t[:, :], ps[:, :], bo_bc[:, ts(dt, 512)])
                nc.sync.dma_start(out=out[ts(qc, P), ts(dt, 512)], in_=o_t[:, :])


_NC_CACHE = {}


def get_nc(variant=None):
    if variant is None:
        variant = os.environ.get("KVARIANT", "")
    if variant not in _NC_CACHE:
        nc = bacc.Bacc("TRN2", target_bir_lowering=False, debug=False,
                       num_devices=N_CORES)
        with tile.TileContext(nc) as tc:
            build_kernel(nc, tc, variant)
        nc.compile()
        _NC_CACHE[variant] = nc
    return _NC_CACHE[variant]


def _bf(a):
    return np.ascontiguousarray(a.astype(ml_dtypes.bfloat16))


def shard_inputs(inputs):
    f = lambda n: np.asarray(inputs[n], dtype=np.float32)
    iq, ik, iv = f("input_query"), f("input_key"), f("input_value")
    wq, wk, wv = f("Wq"), f("Wk"), f("Wv")
    bq, bk, bv = f("bq"), f("bk"), f("bv")
    wo, bo = f("Wo"), f("bo")

    # weights -> [128 d-in-chunk, 8 dchunk, (h k)] bf16
    def pack_w(w):  # [H, D, DK]
        x = np.transpose(w, (1, 0, 2)).reshape(8, P, H * DK)  # [dc, dp, (h k)]
        return _bf(np.transpose(x, (1, 0, 2)))  # [128, 8, H*DK]

    # biases -> [128 pair-stacked, 8 hp] f32
    def pack_b(b):  # [H, DK]
        x = b.reshape(NHP, 2 * DK).T  # [128, NHP]
        return np.ascontiguousarray(x)

    shared = {
        "wq": pack_w(wq), "wk": pack_w(wk), "wv": pack_w(wv),
        "bq": pack_b(bq), "bk": pack_b(bk),
        "bvr": _bf(bv.reshape(1, H * DV)),
        "wo": _bf(np.transpose(wo.reshape(8, P, D), (1, 0, 2))),
        "bo": np.ascontiguousarray(bo[None, :]),
    }
    in_maps = []
    for c in range(N_CORES):
        b_, half = c // 2, c % 2
        m = dict(shared)
        m["xqT"] = _bf(iq[b_, half * SQ:(half + 1) * SQ, :].T)
        m["xkT"] = _bf(ik[b_].T)
        m["xvT"] = _bf(iv[b_].T)
        in_maps.append(m)
    return in_maps


def kernel(**inputs):
    nc = get_nc()
    in_maps = shard_inputs(inputs)
    res = run_bass_kernel_spmd(nc, in_maps, core_ids=list(range(N_CORES)),
                               trace=False)
    out = np.empty((B, S, D), np.float32)
    for c in range(N_CORES):
        b_, half = c // 2, c % 2
        out[b_, half * SQ:(half + 1) * SQ, :] = res.results[c]["out"]
    return out

